# revision 1
# baseline (speedup 1.0000x reference)
"""Fused LayerNorm + MHA + out-proj for Trainium2, SPMD across 8 NeuronCores.

Problem: x[2,2048,1024] -> LN -> qkv (w_qkv[1024,3072]) -> 16-head attention
(dim_head 64) -> out proj (w_out[1024,1024] + b_out).

Sharding: core c handles batch c//4 and head-quad c%4 (heads 4*(c%4)..+4).
Each core: LN + transpose of its batch (replicated within the batch group),
qkv for its 4 heads, full attention for its 4 (b,h) pairs, then an 8-way
AllToAll redistributes head outputs: core c owns output rows [256c, 256c+256)
of EACH batch, so every core's chunk addressing is identical (SPMD-safe).
Each core then computes the final projection for its 512 rows locally.

Key techniques: feature-major (transposed) data flow end-to-end; fp32r
(rounded-fp32, full-rate) matmuls for LN-transpose/qkv/scores; softmax exp on
ScalarE straight out of PSUM with the 1/sqrt(d) scale folded into the
activation; no max-subtraction (scores are O(5) sigma, exp is safe in fp32);
softmax denominator produced by a fused ones-column in the attn@v stationary
operand (psum row 0 = sum of exp, rows 64:128 = head outputs, 64-aligned for
engine access); denominators broadcast across partitions on the otherwise
idle GpSimd engine; bf16 for exp-weights/v/out-projection; per-head-pair
collectives so the first AllToAll overlaps the second half of attention.
"""
import sys
sys.path.insert(0, '/opt/trn_rl_repo')
import numpy as np

import concourse.bass as bass
import concourse.tile as tile
import concourse.mybir as mybir
from concourse import bacc
from concourse.bass_utils import run_bass_kernel_spmd
from concourse.masks import make_identity

F32 = mybir.dt.float32
F32R = mybir.dt.float32r
BF16 = mybir.dt.bfloat16
AF = mybir.ActivationFunctionType
ALU = mybir.AluOpType

N_CORES = 8
B, N, DIM = 2, 2048, 1024
HEADS, DHEAD = 16, 64
H_LOC = 4                    # heads per core
ROWS = N                     # rows per core (one batch)
DT = DIM // 128              # 8 dim tiles
RCHUNK = 512
N_CH = ROWS // RCHUNK        # 4 row chunks
NKT = N // 128               # 16 key tiles
NQC = N // 512               # 4 query chunks
SCALE = DHEAD ** -0.5
EPS = 1e-5
# bf16-space Schraudolph fast exp: bitcast_bf16(int16(s*A + B)) ~ exp(SCALE*s)
A_SCH = SCALE * 128.0 / float(np.log(2.0))
B_SCH = 127.0 * 128.0 - 7.5
I16 = mybir.dt.int16

_CACHED_NC = None


def build():
    nc = bacc.Bacc("TRN2", target_bir_lowering=False, debug=False,
                   num_devices=N_CORES)
    x_ext = nc.dram_tensor("x", [ROWS, DIM], F32, kind="ExternalInput")
    gamma_ext = nc.dram_tensor("gamma", [DIM], F32, kind="ExternalInput")
    beta_ext = nc.dram_tensor("beta", [DIM], F32, kind="ExternalInput")
    wqkv_ext = nc.dram_tensor("wqkv", [DIM, 3 * H_LOC * DHEAD], F32,
                              kind="ExternalInput")
    wout_ext = nc.dram_tensor("wout", [DIM, DIM], F32, kind="ExternalInput")
    bout_ext = nc.dram_tensor("bout", [DIM], F32, kind="ExternalInput")
    out_ext = nc.dram_tensor("out", [RCHUNK, DIM], F32, kind="ExternalOutput")

    with tile.TileContext(nc) as tc:
        with tc.tile_pool(name="singles", bufs=1) as singles, \
             tc.tile_pool(name="xin", bufs=4) as xin, \
             tc.tile_pool(name="xc", bufs=6) as xcp, \
             tc.tile_pool(name="lnxt", bufs=1) as lnxtp, \
             tc.tile_pool(name="stats", bufs=8) as statsp, \
             tc.tile_pool(name="vt", bufs=2) as vtp, \
             tc.tile_pool(name="exps", bufs=5) as expsp, \
             tc.tile_pool(name="div", bufs=2) as divp, \
             tc.tile_pool(name="osb", bufs=2) as osbp, \
             tc.tile_pool(name="dram", bufs=1, space="DRAM") as dram:

            # ---------------- constants / weights ----------------
            ident_f = singles.tile([128, 128], F32)
            make_identity(nc, ident_f)
            ident = singles.tile([128, 128], BF16)
            nc.vector.tensor_copy(ident, ident_f)

            # gamma/beta as [128, DT] (partition p, dim tile dt -> dim dt*128+p)
            gamma_sb = singles.tile([128, DT], F32)
            beta_sb = singles.tile([128, DT], F32)
            nc.sync.dma_start(out=gamma_sb,
                              in_=gamma_ext.ap().rearrange("(dt p) -> p dt", p=128))
            nc.sync.dma_start(out=beta_sb,
                              in_=beta_ext.ap().rearrange("(dt p) -> p dt", p=128))

            eps_sb = singles.tile([128, 1], F32)
            nc.vector.memset(eps_sb, EPS)

            # w_qkv slice as bf16 lhsT tiles via SWDGE cast DMA (the
            # dma itself is emitted after chunk 0's x loads so the input
            # stream, which gates the LayerNorm start, goes first on the
            # shared SWDGE queue; the weights are not needed until the
            # first qkv matmul ~20us later)
            w_sb = singles.tile([128, DT, 3 * H_LOC * DHEAD], BF16)
            # w_out bf16 lhsT tiles (load emitted inside phase-1 chunk 0)
            wo_sb = singles.tile([128, DT, DIM], BF16)

            # b_out bf16 [1, 1024] + ones lhsT [1, 128] for broadcast-add
            # (load emitted in phase-1 chunk 0, after the x input stream)
            bo_sb = singles.tile([1, DIM], BF16)
            ones_bf = singles.tile([1, 128], BF16)
            nc.vector.memset(ones_bf, 1.0)

            # persistent activations
            # bf16 q/k: halves the qkv ACT-drain bytes and SBUF footprint;
            # score noise from bf16 operands is normalized away by softmax
            qT = [singles.tile([128, ROWS], BF16, name=f"qT{i}") for i in range(2)]
            kT = [singles.tile([128, ROWS], BF16, name=f"kT{i}") for i in range(2)]
            # v_aug[:, h, t, 0]=1.0 (softmax denom), cols 1:64 zero pad,
            # [:, h, t, 64:128]=v -> attn@v psum row 0 = denom, rows 64:128 =
            # head outputs (64-aligned partition base for DVE access)
            v_aug = singles.tile([128, H_LOC, NKT, 128], BF16)
            # DVE memsets: keep the gpsimd/SWDGE queue free for the x
            # input loads that now share it (bf16 cast DMA)
            nc.vector.memset(v_aug[:, :, :, 0:64], 0.0)
            nc.vector.memset(v_aug[:, :, :, 0:1], 1.0)
            # head outputs (transposed): pair X holds heads 2X, 2X+1 in free dim
            oh = [singles.tile([128, 2, ROWS], BF16, name=f"oh{i}") for i in range(2)]

            # ---------------- phase 1: LN + transpose + qkv ----------------
            # one pool spans phases 1+2: tag trsc(3x2 banks) + ov(2x1) = 8
            ps1_cm = tc.tile_pool(name="ps1", bufs=3, space="PSUM")
            ps_ov_cm = tc.tile_pool(name="psov", bufs=2, space="PSUM")
            ps1 = ps1_cm.__enter__(); ps_ov = ps_ov_cm.__enter__()
            for ch in range(N_CH):
                xcs = []
                for rt in range(4):
                    r0 = ch * RCHUNK + rt * 128
                    # bf16 x via SWDGE cast DMA: halves the 8MB input stream
                    # and doubles DVE throughput for bn_stats/centering
                    x_t = xin.tile([128, DIM], BF16)
                    nc.gpsimd.dma_start(out=x_t,
                                        in_=x_ext.ap()[r0:r0 + 128, :])
                    st = statsp.tile([128, 2, 6], F32, tag="bn")
                    for sg in range(2):
                        nc.vector.bn_stats(out=st[:, sg, :],
                                           in_=x_t[:, sg * 512:(sg + 1) * 512])
                    mv = statsp.tile([128, 2], F32, tag="mv")
                    nc.vector.bn_aggr(out=mv, in_=st)
                    rstd = statsp.tile([128, 1], F32, tag="rstd")
                    nc.scalar.activation(out=rstd, in_=mv[:, 1:2], func=AF.Sqrt,
                                         bias=eps_sb, scale=1.0)
                    nc.vector.reciprocal(out=rstd, in_=rstd)
                    xc = xcp.tile([128, DIM], BF16)
                    with nc.allow_low_precision(reason="bf16 activations"):
                        nc.vector.tensor_scalar(out=xc, in0=x_t,
                                                scalar1=mv[:, 0:1],
                                                scalar2=rstd,
                                                op0=ALU.subtract,
                                                op1=ALU.mult)
                    xcs.append(xc)

                if ch == 0:
                    nc.gpsimd.dma_start(
                        out=w_sb,
                        in_=wqkv_ext.ap().rearrange("(dt p) c -> p dt c",
                                                    p=128))
                    # w_out load emitted here too: its 4MB HBM read streams
                    # during compute-heavy phase 1 instead of contending
                    # with AllToAll traffic near the phase-2/3 boundary
                    nc.gpsimd.dma_start(
                        out=wo_sb,
                        in_=wout_ext.ap().rearrange("(it p) c -> p it c",
                                                    p=128))
                    nc.gpsimd.dma_start(out=bo_sb,
                                        in_=bout_ext.ap().unsqueeze(0))

                lnxt = lnxtp.tile([128, DT, RCHUNK], BF16)
                for db in range(DT):
                    tr_ps = ps1.tile([128, RCHUNK], BF16, tag="trsc")
                    for rt in range(4):
                        nc.tensor.transpose(tr_ps[:, rt * 128:(rt + 1) * 128],
                                            xcs[rt][:, db * 128:(db + 1) * 128],
                                            ident)
                    # LN affine (gamma, beta are per-partition here); on ACT
                    # to keep DVE off the PE critical path
                    with nc.allow_low_precision(reason="bf16 activations"):
                        nc.scalar.activation(out=lnxt[:, db, :], in_=tr_ps,
                                             func=AF.Identity,
                                             bias=beta_sb[:, db:db + 1],
                                             scale=gamma_sb[:, db:db + 1])

                # qkv matmuls: single-bank accumulation passes
                for X in range(2):
                    sl = slice(ch * RCHUNK, (ch + 1) * RCHUNK)
                    vt = vtp.tile([128, RCHUNK], BF16)
                    qkv_dst = [kT[X][:, sl], qT[X][:, sl], vt]
                    for j, jc in enumerate([1, 0, 2]):  # k first, then q, v
                        ct = jc * 2 + X
                        qkv_ps = ps1.tile([128, RCHUNK], F32, tag="trsc",
                                          name=f"qkv_ps_{ch}_{X}_{j}")
                        for db in range(DT):
                            nc.tensor.matmul(
                                qkv_ps,
                                w_sb[:, db, ct * 128:(ct + 1) * 128],
                                lnxt[:, db, :],
                                start=(db == 0), stop=(db == DT - 1))
                        with nc.allow_low_precision(reason="bf16 q/k"):
                            (nc.scalar.copy if j < 2
                             else nc.vector.tensor_copy)(qkv_dst[j], qkv_ps)
                    # transpose v to normal layout, split into v_aug tiles
                    for blk in range(4):
                        t = ch * 4 + blk
                        v_ps = ps1.tile([128, 128], BF16, tag="trsc")
                        nc.tensor.transpose(v_ps, vt[:, blk * 128:(blk + 1) * 128],
                                            ident)
                        nc.vector.tensor_copy(
                            v_aug[:, 2 * X:2 * X + 2, t, 64:128],
                            v_ps.rearrange("p (h d) -> p h d", h=2))

            # ---------------- phase 2: attention ----------------
            a2a_in = [dram.tile([8, 2, 64, 256], BF16, name=f"a2a_in{i}")
                      for i in range(2)]
            a2a_out = [dram.tile([8, 128, 256], BF16, name=f"a2a_out{i}")
                       for i in range(2)]

            # outT tiles declared up front; each pair's assembly DMA is
            # emitted right after its collective so outT0 consumers don't
            # queue behind A2A#2 on the DMA semaphore
            outT = [singles.tile([128, DT // 2, 2, 256], BF16, name=f"outT{i}")
                    for i in range(2)]


            def divide_pre(o_ps):
                # reciprocal of the fused denominator row + gpsimd broadcast
                # across partitions; emitted right after the chunk's attnv
                rbs = []
                for i in range(2):
                    # bf16 reciprocal/broadcast: halves the bytes the slow
                    # software gpsimd engine must replicate across partitions
                    r1 = divp.tile([1, 512], BF16, tag="r1", bufs=4)
                    with nc.allow_low_precision(reason="bf16 denom"):
                        nc.vector.reciprocal(out=r1, in_=o_ps[i][0:1, :])
                    rb = divp.tile([128, 512], BF16, tag="rb", bufs=4)
                    nc.gpsimd.partition_broadcast(out_ap=rb, in_ap=r1)
                    rbs.append(rb)
                return rbs

            def divide_mult(X, qc, o_ps, rbs):
                # the multiply is deferred a full chunk so the in-order DVE
                # queue never stalls waiting on the (slow, ~3-5us) gpsimd
                # broadcast -- that stall would delay the chunk's DVE
                # Schraudolph-exp work and cascade into PE idle time
                qsl = slice(qc * 512, (qc + 1) * 512)
                for i in range(2):
                    nc.vector.tensor_tensor(out=oh[X][64:128, i, qsl],
                                            in0=o_ps[i][64:128, :],
                                            in1=rbs[i][64:128, :],
                                            op=ALU.mult)

            pending = None
            for X in range(2):  # head pair (outer: enables split A2A)
                for qc in range(NQC):
                    qsl = slice(qc * 512, (qc + 1) * 512)
                    o_ps = [ps_ov.tile([128, 512], F32, name=f"ov_{X}_{qc}_{i}", tag="ov") for i in range(2)]
                    for tp in range(NKT // 2):  # key-tile pairs
                        s_ps = [ps1.tile([128, 2, 512], F32, name=f"sc_{X}_{qc}_{tp}_{i}", tag="trsc")
                                for i in range(2)]
                        for ti in range(2):
                            t = tp * 2 + ti
                            ksl = slice(t * 128, (t + 1) * 128)
                            nc.tensor.matmul(s_ps[0][:, ti, :],
                                             kT[X][0:64, ksl], qT[X][0:64, qsl],
                                             start=True, stop=True,
                                             tile_position=(0, 0))
                            nc.tensor.matmul(s_ps[1][:, ti, :],
                                             kT[X][64:128, ksl], qT[X][64:128, qsl],
                                             start=True, stop=True,
                                             tile_position=(64, 0))
                        for i in range(2):  # head within pair
                            h = 2 * X + i
                            if (2 * tp + i) % 2 == 1:
                                # one-op DVE fast exp (Schraudolph, bf16
                                # space): int16 output bits bitcast to bf16
                                # are exp(SCALE*s); offloads 3/8 of the exp
                                # work from the saturated ScalarE
                                ex_i = expsp.tile([128, 2, 512], I16,
                                                  tag="exi")
                                nc.vector.tensor_scalar(
                                    out=ex_i, in0=s_ps[i],
                                    scalar1=A_SCH, scalar2=B_SCH,
                                    op0=ALU.mult, op1=ALU.add)
                                ex = ex_i.bitcast(BF16)
                            else:
                                ex = expsp.tile([128, 2, 512], BF16)
                                nc.scalar.activation(out=ex, in_=s_ps[i],
                                                     func=AF.Exp,
                                                     bias=0.0, scale=SCALE)
                            for ti in range(2):
                                t = tp * 2 + ti
                                nc.tensor.matmul(o_ps[i],
                                                 v_aug[:, h, t, 0:128],
                                                 ex[:, ti, :],
                                                 start=(t == 0), stop=(t == NKT - 1))
                    rbs = divide_pre(o_ps)
                    if pending is not None:
                        divide_mult(*pending)
                    pending = (X, qc, o_ps, rbs)

                # flush the deferred multiply before this pair's A2A (the
                # A2A input DMAs read oh)
                divide_mult(*pending)
                pending = None

                # A2A for this head pair: chunk j = my batch's rows
                # [256j, 256j+256) for core j; X=0's collective overlaps
                # X=1's attention compute.
                if X == 1:
                    # assemble pair-0 outT before the second collective is
                    # issued (Tile keeps collective-adjacent order)
                    for H in range(2):
                        nc.sync.dma_start(
                            out=outT[0][:, :, H, :],
                            in_=a2a_out[0][4 * H:4 * (H + 1)].rearrange(
                                "q p r -> p q r"))
                for j in range(8):
                    nc.sync.dma_start(
                        out=a2a_in[X][j, :, :, :].rearrange("i d r -> d i r"),
                        in_=oh[X][64:128, :, j * 256:(j + 1) * 256])
                nc.gpsimd.collective_compute(
                    "AllToAll", ALU.bypass,
                    replica_groups=[[0, 1, 2, 3, 4, 5, 6, 7]],
                    ins=[a2a_in[X].opt()], outs=[a2a_out[X].opt()])
            for H in range(2):
                # SWDGE queue (separate DMA semaphore) so the pair-0
                # out-proj matmuls' HWDGE wait threshold excludes these
                nc.gpsimd.dma_start(
                    out=outT[1][:, :, H, :],
                    in_=a2a_out[1][4 * H:4 * (H + 1)].rearrange(
                        "q p r -> p q r"))


            # ---------------- phase 3: outT assembly + out proj ----------
            # a2a_out[X] slot j' = core j' (batch j'//4, head-quad j'%4)
            # inner slice (pair X) for my 256 rows of THEIR batch.
            # inner tile it = q*2 + X where slot j' = H*4 + q.


            ps_ov_cm.__exit__(None, None, None); ps1_cm.__exit__(None, None, None)
            # all 8 output accumulation groups live at once (8 banks) so
            # EVERY pair-0 matmul precedes any pair-1 matmul in the PE
            # stream: the whole first pass overlaps the second AllToAll
            # (the in-order PE queue would otherwise stall group 2's pair-0
            # work behind group 1's pair-1 wait).
            ps_op_cm = tc.tile_pool(name="ps_op", bufs=2, space="PSUM")
            ps_op = ps_op_cm.__enter__()
            # pass 1: pair-0 inner tiles + bias as CLOSED accumulation
            # groups, partials drained to SBUF — accumulation groups
            # schedule as contiguous units, so a group containing a pair-1
            # matmul would drag its pair-0 work behind the second AllToAll.
            e_parts = {}
            for rt in range(4):
                for oc in range(2):
                    ep = ps_op.tile([128, 512], F32, tag="op",
                                    name=f"ep_{rt}_{oc}")
                    for q in range(DT // 2):
                        nc.tensor.matmul(
                            ep,
                            outT[0][:, q, rt // 2,
                                    (rt % 2) * 128:(rt % 2) * 128 + 128],
                            wo_sb[:, q * 2, oc * 512:(oc + 1) * 512],
                            start=(q == 0), stop=False)
                    nc.tensor.matmul(
                        ep, ones_bf, bo_sb[:, oc * 512:(oc + 1) * 512],
                        start=False, stop=True)
                    e_sb = osbp.tile([128, 512], BF16, tag="e_sb", bufs=8,
                                     name=f"e_sb_{rt}_{oc}")
                    nc.vector.tensor_copy(e_sb, ep)
                    e_parts[(rt, oc)] = e_sb
            # pass 2: pair-1 tiles into fresh groups; the combining add
            # replaces the drain copy
            for rt in range(4):
                for oc in range(2):
                    op_ps = ps_op.tile([128, 512], F32, tag="op",
                                       name=f"op_ps_{rt}_{oc}")
                    for q in range(DT // 2):
                        nc.tensor.matmul(
                            op_ps,
                            outT[1][:, q, rt // 2,
                                    (rt % 2) * 128:(rt % 2) * 128 + 128],
                            wo_sb[:, q * 2 + 1, oc * 512:(oc + 1) * 512],
                            start=(q == 0), stop=(q == DT // 2 - 1))
                    o_sb = osbp.tile([128, 512], F32, tag="o_sb")
                    nc.vector.tensor_tensor(out=o_sb, in0=op_ps,
                                            in1=e_parts[(rt, oc)],
                                            op=ALU.add)
                    nc.sync.dma_start(
                        out=out_ext.ap()[rt * 128:(rt + 1) * 128,
                                         oc * 512:(oc + 1) * 512], in_=o_sb)
            ps_op_cm.__exit__(None, None, None)

    nc.compile()
    return nc


def _make_in_maps(inputs):
    x = np.ascontiguousarray(
        np.asarray(inputs["x"], dtype=np.float32).reshape(B * N, DIM))
    gamma = np.asarray(inputs["gamma"], dtype=np.float32)
    beta = np.asarray(inputs["beta"], dtype=np.float32)
    w_qkv = np.asarray(inputs["w_qkv"], dtype=np.float32)
    w_out = np.ascontiguousarray(np.asarray(inputs["w_out"], dtype=np.float32))
    b_out = np.asarray(inputs["b_out"], dtype=np.float32)

    in_maps = []
    for c in range(N_CORES):
        b = c // 4
        qd = c % 4
        cols = []
        for j in range(3):
            cols.append(w_qkv[:, j * DIM + qd * 256:(j * DIM + qd * 256) + 256])
        wqkv_s = np.ascontiguousarray(np.concatenate(cols, axis=1))
        in_maps.append(dict(
            x=np.ascontiguousarray(x[b * N:(b + 1) * N]),
            gamma=gamma, beta=beta,
            wqkv=wqkv_s, wout=w_out, bout=b_out))
    return in_maps


def kernel(x, gamma, beta, w_qkv, w_out, b_out):
    global _CACHED_NC
    if _CACHED_NC is None:
        _CACHED_NC = build()
    nc = _CACHED_NC
    in_maps = _make_in_maps(dict(x=x, gamma=gamma, beta=beta, w_qkv=w_qkv,
                                 w_out=w_out, b_out=b_out))
    res = run_bass_kernel_spmd(nc, in_maps, core_ids=list(range(N_CORES)))
    # core c's "out" [512, 1024] = rows [256c, 256c+256) of batch 0 then batch 1
    out = np.empty((B, N, DIM), dtype=np.float32)
    for c in range(N_CORES):
        o = res.results[c]["out"]
        out[0, 256 * c:256 * (c + 1)] = o[0:256]
        out[1, 256 * c:256 * (c + 1)] = o[256:512]
    return out

